# revision 21
# baseline (speedup 1.0000x reference)
"""Trainium2 Bass kernel for nn_AttentionHead (B=8, N=2048, D=512, d=64).

Reference semantics (faithful to the torch original):
    K = key_input   @ W_key        # note: W_key used for Q, K AND V
    Q = query_input @ W_key
    V = value_input @ W_key
    S = Q @ K^T / sqrt(512)        # scaled by INPUT dim, not head dim
    S = mask(padding), causal-mask if masked_attention
    out = softmax(S) @ V

Sharding: pure data parallelism over batch — core b computes batch element b.
No collectives. Host-side prep is layout only (transpose + dtype cast +
output unpermute); every FLOP of the math runs on-device.

Device algorithm (per core), v4:
  - xq/xk stream in fp8e4 (W_qk prescaled x16 host-side, descale folded into
    the exp scale); projections use DoubleRow perf mode (256-deep
    contraction, half the matmuls); xv stays bf16 to protect output precision
  - host packs inputs in the exact SBUF tile layout so every DMA line is
    contiguous; DMA transfers are sequenced into priority waves (q0k0 ->
    q1k1 -> v0 -> q2k2 -> v1 -> q3k3 -> v2 -> v3) via tiny SBUF->SBUF "gate"
    DMAs — the rings round-robin across all in-flight transfers, so issue
    order alone gives no priority
  - q-block-outer attention: per 512-wide q-block, k-chunks processed in
    row-packed pairs (two 64-deep S matmuls concurrently in disjoint PE row
    groups) writing one [128, 1024] PSUM tile; ONE wide exp per pair on ACT
    (amortizes the ~293ns ACTIVATE overhead); p_sb pool sized one-per-group
    so no recycle deps serialize the ACT queue
  - diagonal k-chunks compute S full-width (free under row-pairing) so the
    wide exp read is contiguous; only 128-wide diagonal blocks get the
    upper-triangular mask multiply (DVE early, gpsimd for late q-blocks);
    PV uses exact causal widths
  - O.T [65, q] accumulated in PSUM over k-chunks (ones column appended to
    V-natural gives softmax denominators as row 64); per-q-block epilogue:
    PE-transpose, reciprocal, then 4 independent scale->DMA chains
  - PSUM: 2x [128,1024] S (4 banks) + 2x [65,512] O (2) + 2x [128,512]
    proj/transpose (2) = 8 banks
"""

import math

import numpy as np
import ml_dtypes

import concourse.bass as bass
import concourse.tile as tile
from concourse import bacc, mybir
from concourse import masks
from concourse.bass_utils import run_bass_kernel_spmd

P = 128            # partitions / k-chunk size
N = 2048           # sequence length
D = 512            # embedding dim
DH = 64            # head dim
EC = D // P        # 4 e-chunks for the (bf16) V projection contraction
EC2 = D // (2 * P)  # 2 double-row chunks for the fp8 Q/K projections
KC = N // P        # 16 k-chunks
QW = 512           # q block width
NQB = N // QW      # 4 q blocks / n slices
WS = 16.0          # host-side W_qk prescale (fp8 range use)
SCALE = 1.0 / math.sqrt(float(D))
EXP_SCALE = SCALE / (WS * WS)

BF16 = mybir.dt.bfloat16
FP8 = mybir.dt.float8e4
F32 = mybir.dt.float32
DR = mybir.MatmulPerfMode.DoubleRow

_BUILD_CACHE = {}

OPTS = {
    "pe_warm": 12,     # dummy matmuls at t=0 to lift the HAM clock gate
    "ppool": 20,       # p_sb wide-tile buffers: one per group, so no recycle
                       # deps ever land as EVENT_SEMAPHORE waits on ACT
    "use_dr": True,    # DoubleRow fp8 projections
    "spool": 3,        # S psum bufs (3x2 banks); o/j pools get 1 bank each
    "opool": 1,
    "jpool": 1,
}


def _ensure_ntff_hook():
    """Install the antenv.axon_hooks shim so trace=True works under axon."""
    try:
        import antenv.axon_hooks  # noqa: F401
        return
    except ImportError:
        pass
    import sys
    import types

    try:
        from trn_agent_boot.trn_boot import _ntff_profile_via_ctypes
        hook = _ntff_profile_via_ctypes("/opt/axon/libaxon_pjrt.so")
    except Exception:
        hook = None
    mod = types.ModuleType("antenv.axon_hooks")
    state = {"hook": hook}
    mod.get_axon_ntff_profile_hook = lambda: state["hook"]
    mod.set_axon_ntff_profile_hook = lambda h: state.update(hook=h)
    sys.modules["antenv.axon_hooks"] = mod
    import antenv

    antenv.axon_hooks = mod


def _build(causal: bool, has_padding: bool):
    nc = bacc.Bacc("TRN2", target_bir_lowering=False, debug=False, num_devices=8)
    use_dr = OPTS["use_dr"]

    # inputs prepacked host-side in SBUF tile layout
    if use_dr:
        xq_d = nc.dram_tensor("xq", [NQB * P, EC2, 2, QW], FP8, kind="ExternalInput")
        xk_d = nc.dram_tensor("xk", [NQB * P, EC2, 2, QW], FP8, kind="ExternalInput")
        wqk_d = nc.dram_tensor("wqk", [P, EC2, 2, 2 * DH], FP8, kind="ExternalInput")
    else:
        xq_d = nc.dram_tensor("xq", [NQB * P, EC, QW], FP8, kind="ExternalInput")
        xk_d = nc.dram_tensor("xk", [NQB * P, EC, QW], FP8, kind="ExternalInput")
        wqk_d = nc.dram_tensor("wqk", [P, EC, 2 * DH], FP8, kind="ExternalInput")
    xv_d = nc.dram_tensor("xv", [NQB * P, EC, QW], BF16, kind="ExternalInput")
    wv_d = nc.dram_tensor("wv", [P, EC, DH], BF16, kind="ExternalInput")
    if has_padding:
        km_d = nc.dram_tensor("kmask", [KC, P], F32, kind="ExternalInput")
    # out rows = qb*128 + p, col block i -> full row q = qb*512 + i*128 + p
    # (host unpermutes); per-partition DMA lines are contiguous
    out_d = nc.dram_tensor("out", [NQB * P, NQB * DH], F32, kind="ExternalOutput")

    with tile.TileContext(nc) as tc:
        with (
            tc.tile_pool(name="const", bufs=1) as cpool,
            tc.tile_pool(name="x", bufs=12) as xpool,
            tc.tile_pool(name="big", bufs=1) as bigpool,
            tc.tile_pool(name="p", bufs=OPTS["ppool"]) as ppool,
            tc.tile_pool(name="epi", bufs=2) as epipool,
            tc.tile_pool(name="osb", bufs=2) as opool_sb,
            tc.tile_pool(name="o", bufs=OPTS["opool"], space="PSUM") as opool,
            tc.tile_pool(name="s", bufs=OPTS["spool"], space="PSUM") as spool,
            tc.tile_pool(name="j", bufs=OPTS["jpool"], space="PSUM") as jpool,
        ):
            # --- ACT warmup: load the exp table during the DMA window ---
            warm = cpool.tile([P, 1], F32)
            nc.vector.memset(warm[:], 0.0)
            nc.scalar.activation(warm[:], warm[:], mybir.ActivationFunctionType.Exp)

            # consts emitted before any DMA issue so their engines (vector
            # memset, gpsimd affine_select) aren't stuck behind dma issues
            wjunk = cpool.tile([P, P], BF16)
            nc.vector.memset(wjunk[:], 0.25)
            ident = cpool.tile([P, P], F32)
            masks.make_identity(nc, ident[:])
            tri = cpool.tile([P, P], BF16)
            masks.make_upper_triangular(nc, tri[:], val=1.0, diag=True)

            # --- weights + x tiles ---
            if use_dr:
                wqk_sb = cpool.tile([P, EC2, 2, 2 * DH], FP8)
            else:
                wqk_sb = cpool.tile([P, EC, 2 * DH], FP8)
            wv_sb = cpool.tile([P, EC, DH], BF16)
            if has_padding:
                km_sb = cpool.tile([P, KC], F32)
                nc.sync.dma_start(km_sb[:], km_d.ap().transpose([1, 0]))

            xq_sb, xk_sb, xv_sb = {}, {}, {}

            def alloc_qk(s):
                shp = [P, EC2, 2, QW] if use_dr else [P, EC, QW]
                xq_sb[s] = xpool.tile(shp, FP8, tag="x", name=f"xq{s}")
                xk_sb[s] = xpool.tile(shp, FP8, tag="x", name=f"xk{s}")

            def alloc_v(s):
                xv_sb[(s, 0)] = xpool.tile([P, EC // 2, QW], BF16, tag="x",
                                           name=f"xv{s}a")
                xv_sb[(s, 1)] = xpool.tile([P, EC // 2, QW], BF16, tag="x",
                                           name=f"xv{s}b")

            def dma_x(eng, t, dram, s, half=None):
                rows = dram.ap()[s * P:(s + 1) * P]
                if half is None:
                    eng.dma_start(t[:], rows)
                else:
                    eng.dma_start(
                        t[:], rows[:, half * (EC // 2):(half + 1) * (EC // 2), :]
                    )

            def corner(t):
                nd = len(t.shape)
                return t[tuple([slice(0, 1)] * (nd - 1) + [slice(0, 16)])]

            def latch(src_tile, dst_tiles):
                """Force dst tiles' DMA starts to wait until src_tile's DMA
                has fully landed: a tiny vector copy reads src's corner and
                writes each dst's corner — the dst DMA (full-tile write)
                then has a WAW dep on the copy. This is the only reliable
                transfer prioritization: the rings round-robin across all
                in-flight DMAs, and the tile scheduler reorders plain issue
                order."""
                for dt in dst_tiles:
                    nc.vector.tensor_copy(corner(dt), corner(src_tile))

            # DMA waves: q0 k0 -> q1 k1 -> v0 (+wv) -> v1 -> v2 -> v3;
            # q2/k2, q3/k3 ride on x-pool buf reuse (gated on proj reads).
            alloc_qk(0)
            alloc_qk(1)
            alloc_v(0)
            alloc_v(1)
            alloc_v(2)
            alloc_v(3)
            nc.scalar.dma_start(wqk_sb[:], wqk_d.ap())
            dma_x(nc.scalar, xq_sb[0], xq_d, 0)
            dma_x(nc.sync, xk_sb[0], xk_d, 0)
            latch(xq_sb[0], [xq_sb[1], xk_sb[1]])
            dma_x(nc.sync, xq_sb[1], xq_d, 1)
            dma_x(nc.gpsimd, xk_sb[1], xk_d, 1)
            latch(xq_sb[1], [xv_sb[(0, 0)], xv_sb[(0, 1)], wv_sb])
            dma_x(nc.sync, xv_sb[(0, 0)], xv_d, 0, half=0)
            dma_x(nc.gpsimd, xv_sb[(0, 1)], xv_d, 0, half=1)
            nc.gpsimd.dma_start(wv_sb[:], wv_d.ap())
            latch(xv_sb[(0, 0)], [xv_sb[(1, 0)], xv_sb[(1, 1)]])
            dma_x(nc.sync, xv_sb[(1, 0)], xv_d, 1, half=0)
            dma_x(nc.gpsimd, xv_sb[(1, 1)], xv_d, 1, half=1)
            latch(xv_sb[(1, 0)], [xv_sb[(2, 0)], xv_sb[(2, 1)]])
            dma_x(nc.sync, xv_sb[(2, 0)], xv_d, 2, half=0)
            dma_x(nc.gpsimd, xv_sb[(2, 1)], xv_d, 2, half=1)
            latch(xv_sb[(2, 0)], [xv_sb[(3, 0)], xv_sb[(3, 1)]])
            dma_x(nc.sync, xv_sb[(3, 0)], xv_d, 3, half=0)
            dma_x(nc.gpsimd, xv_sb[(3, 1)], xv_d, 3, half=1)

            # --- PE warmup: HAM clock-gates the PE array to 1.2 GHz until
            # ~3.4us of sustained matmul activity ---
            if OPTS["pe_warm"]:
                wps = jpool.tile([P, QW], F32, tag="j", name="warmps")
                for _ in range(OPTS["pe_warm"]):
                    nc.tensor.matmul(
                        wps[:, :P], wjunk[:], wjunk[:],
                        start=True, stop=True, skip_group_check=True,
                    )

            qt = bigpool.tile([P, N], BF16, tag="qt")   # rows 0-63 QT, 64-127 dup
            kt = bigpool.tile([P, N], BF16, tag="kt")
            vt = bigpool.tile([DH, N], F32, tag="vt")
            v_sb = bigpool.tile([P, KC, DH + 1], BF16, tag="vn")

            def proj_qk(s):
                sl = slice(s * QW, (s + 1) * QW)
                for tname, x_t, big in (("q", xq_sb[s], qt), ("k", xk_sb[s], kt)):
                    ps = jpool.tile([P, QW], F32, tag="j", name=f"{tname}p{s}")
                    if use_dr:
                        for c in range(EC2):
                            nc.tensor.matmul(
                                ps[:],
                                wqk_sb[:, c],
                                x_t[:, c],
                                start=(c == 0),
                                stop=(c == EC2 - 1),
                                perf_mode=DR,
                            )
                    else:
                        for c in range(EC):
                            nc.tensor.matmul(
                                ps[:],
                                wqk_sb[:, c, :],
                                x_t[:, c, :],
                                start=(c == 0),
                                stop=(c == EC - 1),
                            )
                    nc.vector.tensor_copy(big[:, sl], ps[:])

            def proj_v(s):
                sl = slice(s * QW, (s + 1) * QW)
                ps = jpool.tile([P, QW], F32, tag="j", name=f"vp{s}")
                for c in range(EC):
                    nc.tensor.matmul(
                        ps[:DH, :],
                        wv_sb[:, c, :],
                        xv_sb[(s, c // 2)][:, c % 2, :],
                        start=(c == 0),
                        stop=(c == EC - 1),
                    )
                nc.vector.tensor_copy(vt[:, sl], ps[:DH, :])
                # V natural tiles: PE transpose + ones column (row-sums of P
                # come free as row 64 of the PV matmul)
                vtp = jpool.tile([P, NQB, DH + 1], F32, tag="j", name=f"vt{s}")
                for i in range(NQB):
                    j = s * NQB + i
                    nc.tensor.transpose(
                        vtp[:, i, :DH], vt[:, j * P:(j + 1) * P], ident[:DH, :DH]
                    )
                nc.vector.memset(vtp[:, :, DH], 1.0)
                nc.vector.tensor_copy(v_sb[:, s * NQB:(s + 1) * NQB, :], vtp[:])

            # --- attention, q-block outer; k-chunk pairs row-packed ---
            def emit_s_pair(qb, t, p_tiles):
                j0, j1 = 2 * t, 2 * t + 1
                s_ps = spool.tile([P, 2 * QW], F32, tag="s", name=f"s{qb}_{t}")
                # exp reads contiguously from q_off0; j1 computes full width
                # so no unwritten PSUM is read
                q_off0 = max(0, j0 * P - qb * QW) if causal else 0
                nc.tensor.matmul(
                    s_ps[:, q_off0:QW],
                    kt[0:DH, j0 * P:(j0 + 1) * P],
                    qt[0:DH, qb * QW + q_off0:(qb + 1) * QW],
                    start=True, stop=True,
                )
                nc.tensor.matmul(
                    s_ps[:, QW:],
                    kt[DH:P, j1 * P:(j1 + 1) * P],
                    qt[DH:P, qb * QW:(qb + 1) * QW],
                    start=True, stop=True,
                )
                p_sb = ppool.tile([P, 2 * QW], BF16, tag="p", name=f"p{qb}_{t}")
                nc.scalar.activation(
                    p_sb[:, q_off0:],
                    s_ps[:, q_off0:],
                    mybir.ActivationFunctionType.Exp,
                    scale=EXP_SCALE,
                )
                if causal:
                    # late q-blocks' diag masks go to the (by then idle)
                    # gpsimd engine to offload DVE
                    teng = nc.gpsimd if qb >= 2 else nc.vector
                    for idx, j in enumerate((j0, j1)):
                        if j // NQB == qb:
                            # diagonal 128x128 block: keep q_loc >= k_loc
                            lo = idx * QW + (j % NQB) * P
                            teng.tensor_mul(
                                p_sb[:, lo:lo + P], p_sb[:, lo:lo + P], tri[:]
                            )
                if has_padding:
                    for idx, j in enumerate((j0, j1)):
                        off = max(0, j * P - qb * QW) if causal else 0
                        nc.vector.tensor_scalar_mul(
                            p_sb[:, idx * QW + off:(idx + 1) * QW],
                            p_sb[:, idx * QW + off:(idx + 1) * QW],
                            km_sb[:, j:j + 1],
                        )
                p_tiles[t] = p_sb

            def emit_pv(qb, t, o_ps, p_tiles, first, last):
                p_sb = p_tiles.pop(t)
                for idx, j in enumerate((2 * t, 2 * t + 1)):
                    q_off = max(0, j * P - qb * QW) if causal else 0
                    nc.tensor.matmul(
                        o_ps[:, q_off:QW],
                        v_sb[:, j, :],
                        p_sb[:, idx * QW + q_off:(idx + 1) * QW],
                        start=(first and idx == 0),
                        stop=(last and idx == 1),
                    )

            def epilogue(qb, o_ps):
                oT = epipool.tile([DH + 1, QW], F32, tag="ot")
                nc.vector.tensor_copy(oT[:], o_ps[:])
                etp = jpool.tile([P, NQB, DH + 1], F32, tag="j", name=f"et{qb}")
                for i in range(NQB):
                    nc.tensor.transpose(
                        etp[:, i, :], oT[:, i * P:(i + 1) * P],
                        ident[:DH + 1, :DH + 1],
                    )
                recip = epipool.tile([P, NQB], F32, tag="recip")
                nc.vector.reciprocal(recip[:], etp[:, :, DH])
                o_sb = opool_sb.tile([P, NQB, DH], F32, tag="osb",
                                     name=f"osb{qb}")
                for i in range(NQB):
                    nc.vector.tensor_scalar_mul(
                        o_sb[:, i, :], etp[:, i, :DH], recip[:, i:i + 1]
                    )
                nc.sync.dma_start(
                    out_d.ap()[qb * P:(qb + 1) * P, :], o_sb[:]
                )

            # --- main emission: proj interleaved with q-block phases;
            # software-pipelined S/PV so PE work overlaps the wide exps ---
            def attn_qb(qb, npairs, t_projv, reverse):
                order = list(range(npairs))
                if reverse:
                    order.reverse()
                o_ps = opool.tile([DH + 1, QW], F32, tag="o", name=f"o{qb}")
                p_tiles = {}
                for pos, t in enumerate(order):
                    emit_s_pair(qb, t, p_tiles)
                    if t == t_projv:
                        proj_v(qb)
                    if pos > 0:
                        emit_pv(qb, order[pos - 1], o_ps, p_tiles,
                                first=(pos == 1), last=False)
                emit_pv(qb, order[-1], o_ps, p_tiles,
                        first=(npairs == 1), last=True)
                epilogue(qb, o_ps)

            def dma_qk23():
                # slice-2/3 q/k tiles reuse slice-0/1 bufs: their DMA starts
                # are gated on the proj reads just emitted
                alloc_qk(2)
                dma_x(nc.sync, xq_sb[2], xq_d, 2)
                dma_x(nc.gpsimd, xk_sb[2], xk_d, 2)
                alloc_qk(3)
                dma_x(nc.sync, xq_sb[3], xq_d, 3)
                dma_x(nc.gpsimd, xk_sb[3], xk_d, 3)

            if causal:
                proj_qk(0)
                proj_qk(1)
                dma_qk23()
                for qb in range(NQB):
                    npairs = 2 * qb + 2
                    t_projv = npairs - 1 if qb == 0 else 2 * qb
                    # last q-block runs pairs in reverse so its tail after
                    # the final exp is a mask-free full-width PV
                    attn_qb(qb, npairs, t_projv, reverse=(qb == NQB - 1))
                    if qb + 2 < NQB:
                        proj_qk(qb + 2)
            else:
                proj_qk(0)
                proj_qk(1)
                dma_qk23()
                proj_qk(2)
                proj_qk(3)
                for s in range(NQB):
                    proj_v(s)
                for qb in range(NQB):
                    attn_qb(qb, KC // 2, -1, reverse=False)

    nc.compile()
    return nc


def _get(causal: bool, has_padding: bool):
    key = (causal, has_padding)
    if key not in _BUILD_CACHE:
        _BUILD_CACHE[key] = _build(causal, has_padding)
    return _BUILD_CACHE[key]


def _pack_x(x_t: np.ndarray, dtype) -> np.ndarray:
    """[D, N] -> SBUF tile layout [(slice p), chunk, qw]."""
    return np.ascontiguousarray(
        x_t.reshape(EC, P, NQB, QW).transpose(2, 1, 0, 3)
        .reshape(NQB * P, EC, QW).astype(dtype)
    )


def _pack_x_dr(x_t: np.ndarray, dtype) -> np.ndarray:
    """[D, N] -> DoubleRow tile layout [(slice p), c, ko, qw],
    d = c*256 + ko*128 + ki."""
    return np.ascontiguousarray(
        x_t.reshape(EC2, 2, P, NQB, QW).transpose(3, 2, 0, 1, 4)
        .reshape(NQB * P, EC2, 2, QW).astype(dtype)
    )


def run(key_input, query_input, value_input, padding_mask, masked_attention,
        W_key, W_query=None, W_value=None, trace=False, **_ignored):
    key_input = np.asarray(key_input, dtype=np.float32)
    query_input = np.asarray(query_input, dtype=np.float32)
    value_input = np.asarray(value_input, dtype=np.float32)
    padding_mask = np.asarray(padding_mask)
    W_key = np.asarray(W_key, dtype=np.float32)

    B = key_input.shape[0]
    causal = bool(int(np.asarray(masked_attention)))
    has_padding = bool(padding_mask.any())
    nc = _get(causal, has_padding)

    bf = ml_dtypes.bfloat16
    f8 = ml_dtypes.float8_e4m3fn
    wcat = np.concatenate([W_key, W_key], axis=1) * WS
    if OPTS["use_dr"]:
        wqk = np.ascontiguousarray(
            wcat.reshape(EC2, 2, P, 2 * DH).transpose(2, 0, 1, 3).astype(f8)
        )
    else:
        wqk = np.ascontiguousarray(
            wcat.reshape(EC, P, 2 * DH).transpose(1, 0, 2).astype(f8)
        )
    wv = np.ascontiguousarray(
        W_key.reshape(EC, P, DH).transpose(1, 0, 2).astype(bf)
    )
    pack_qk = _pack_x_dr if OPTS["use_dr"] else _pack_x
    in_maps = []
    for b in range(B):
        m = {
            "xq": pack_qk(query_input[b].T, f8),
            "xk": pack_qk(key_input[b].T, f8),
            "xv": _pack_x(value_input[b].T, bf),
            "wqk": wqk,
            "wv": wv,
        }
        if has_padding:
            km = (~padding_mask[b].reshape(N)).astype(np.float32)
            m["kmask"] = np.ascontiguousarray(km.reshape(KC, P))
        in_maps.append(m)

    if trace:
        _ensure_ntff_hook()
    res = run_bass_kernel_spmd(nc, in_maps, core_ids=list(range(B)), trace=trace)
    outs = []
    for b in range(B):
        o = np.asarray(res.results[b]["out"])  # [(qb p), (i d)]
        o = o.reshape(NQB, P, NQB, DH).transpose(0, 2, 1, 3).reshape(N, DH)
        outs.append(o)
    out = np.stack(outs, axis=0)
    return out.astype(np.float32), res


def kernel(**inputs) -> np.ndarray:
    out, _ = run(**inputs)
    return out


# revision 25
# speedup vs baseline: 1.3125x; 1.3125x over previous
"""Trainium2 Bass kernel for nn_AttentionHead (B=8, N=2048, D=512, d=64).

Reference semantics (faithful to the torch original):
    K = key_input   @ W_key        # note: W_key used for Q, K AND V
    Q = query_input @ W_key
    V = value_input @ W_key
    S = Q @ K^T / sqrt(512)        # scaled by INPUT dim, not head dim
    S = mask(padding), causal-mask if masked_attention
    out = softmax(S) @ V

Sharding: pure data parallelism over batch — core b computes batch element b.
No collectives. Host-side prep is layout only (transpose + dtype cast +
output unpermute); every FLOP of the math runs on-device.

Device algorithm (per core), v4:
  - xq/xk stream in fp8e4 (W_qk prescaled x16 host-side, descale folded into
    the exp scale); projections use DoubleRow perf mode (256-deep
    contraction, half the matmuls); xv stays bf16 to protect output precision
  - host packs inputs in the exact SBUF tile layout so every DMA line is
    contiguous; DMA transfers are sequenced into priority waves (q0k0 ->
    q1k1 -> v0 -> q2k2 -> v1 -> q3k3 -> v2 -> v3) via tiny SBUF->SBUF "gate"
    DMAs — the rings round-robin across all in-flight transfers, so issue
    order alone gives no priority
  - q-block-outer attention: per 512-wide q-block, k-chunks processed in
    row-packed pairs (two 64-deep S matmuls concurrently in disjoint PE row
    groups) writing one [128, 1024] PSUM tile; ONE wide exp per pair on ACT
    (amortizes the ~293ns ACTIVATE overhead); p_sb pool sized one-per-group
    so no recycle deps serialize the ACT queue
  - diagonal k-chunks compute S full-width (free under row-pairing) so the
    wide exp read is contiguous; only 128-wide diagonal blocks get the
    upper-triangular mask multiply (DVE early, gpsimd for late q-blocks);
    PV uses exact causal widths
  - O.T [65, q] accumulated in PSUM over k-chunks (ones column appended to
    V-natural gives softmax denominators as row 64); per-q-block epilogue:
    PE-transpose, reciprocal, then 4 independent scale->DMA chains
  - PSUM: 2x [128,1024] S (4 banks) + 2x [65,512] O (2) + 2x [128,512]
    proj/transpose (2) = 8 banks
"""

import math

import numpy as np
import ml_dtypes

import concourse.bass as bass
import concourse.tile as tile
from concourse import bacc, mybir
from concourse import masks
from concourse.bass_utils import run_bass_kernel_spmd

P = 128            # partitions / k-chunk size
N = 2048           # sequence length
D = 512            # embedding dim
DH = 64            # head dim
EC = D // P        # 4 e-chunks for the (bf16) V projection contraction
EC2 = D // (2 * P)  # 2 double-row chunks for the fp8 Q/K projections
KC = N // P        # 16 k-chunks
QW = 512           # q block width
NQB = N // QW      # 4 q blocks / n slices
WS = 16.0          # host-side W_qk prescale (fp8 range use)
SCALE = 1.0 / math.sqrt(float(D))
EXP_SCALE = SCALE / (WS * WS)

BF16 = mybir.dt.bfloat16
FP8 = mybir.dt.float8e4
F32 = mybir.dt.float32
DR = mybir.MatmulPerfMode.DoubleRow

_BUILD_CACHE = {}

OPTS = {
    "pe_warm": 12,     # dummy matmuls at t=0 to lift the HAM clock gate
    "ppool": 20,       # p_sb wide-tile buffers: one per group, so no recycle
                       # deps ever land as EVENT_SEMAPHORE waits on ACT
    "use_dr": True,    # DoubleRow fp8 projections
    "spool": 2,
    "opool": 2,
    "jpool": 2,
}


def _ensure_ntff_hook():
    """Install the antenv.axon_hooks shim so trace=True works under axon."""
    try:
        import antenv.axon_hooks  # noqa: F401
        return
    except ImportError:
        pass
    import sys
    import types

    try:
        from trn_agent_boot.trn_boot import _ntff_profile_via_ctypes
        hook = _ntff_profile_via_ctypes("/opt/axon/libaxon_pjrt.so")
    except Exception:
        hook = None
    mod = types.ModuleType("antenv.axon_hooks")
    state = {"hook": hook}
    mod.get_axon_ntff_profile_hook = lambda: state["hook"]
    mod.set_axon_ntff_profile_hook = lambda h: state.update(hook=h)
    sys.modules["antenv.axon_hooks"] = mod
    import antenv

    antenv.axon_hooks = mod


def _build(causal: bool, has_padding: bool):
    nc = bacc.Bacc("TRN2", target_bir_lowering=False, debug=False, num_devices=8)
    use_dr = OPTS["use_dr"]

    # inputs prepacked host-side in SBUF tile layout
    if use_dr:
        xq_d = nc.dram_tensor("xq", [NQB * P, EC2, 2, QW], FP8, kind="ExternalInput")
        xk_d = nc.dram_tensor("xk", [NQB * P, EC2, 2, QW], FP8, kind="ExternalInput")
        wqk_d = nc.dram_tensor("wqk", [P, EC2, 2, 2 * DH], FP8, kind="ExternalInput")
    else:
        xq_d = nc.dram_tensor("xq", [NQB * P, EC, QW], FP8, kind="ExternalInput")
        xk_d = nc.dram_tensor("xk", [NQB * P, EC, QW], FP8, kind="ExternalInput")
        wqk_d = nc.dram_tensor("wqk", [P, EC, 2 * DH], FP8, kind="ExternalInput")
    xv_d = nc.dram_tensor("xv", [NQB * P, EC, QW], BF16, kind="ExternalInput")
    wv_d = nc.dram_tensor("wv", [P, EC, DH], BF16, kind="ExternalInput")
    if has_padding:
        km_d = nc.dram_tensor("kmask", [KC, P], F32, kind="ExternalInput")
    # out rows = qb*128 + p, col block i -> full row q = qb*512 + i*128 + p
    # (host unpermutes); per-partition DMA lines are contiguous
    out_d = nc.dram_tensor("out", [NQB * P, NQB * DH], F32, kind="ExternalOutput")

    with tile.TileContext(nc) as tc:
        with (
            tc.tile_pool(name="const", bufs=1) as cpool,
            tc.tile_pool(name="x", bufs=4) as xpool,
            tc.tile_pool(name="big", bufs=1) as bigpool,
            tc.tile_pool(name="p", bufs=OPTS["ppool"]) as ppool,
            tc.tile_pool(name="epi", bufs=2) as epipool,
            tc.tile_pool(name="osb", bufs=2) as opool_sb,
            tc.tile_pool(name="o", bufs=OPTS["opool"], space="PSUM") as opool,
            tc.tile_pool(name="s", bufs=OPTS["spool"], space="PSUM") as spool,
            tc.tile_pool(name="j", bufs=OPTS["jpool"], space="PSUM") as jpool,
        ):
            # --- ACT warmup: load the exp table during the DMA window ---
            warm = cpool.tile([P, 1], F32)
            nc.vector.memset(warm[:], 0.0)
            nc.scalar.activation(warm[:], warm[:], mybir.ActivationFunctionType.Exp)

            # consts emitted before any DMA issue so their engines (vector
            # memset, gpsimd affine_select) aren't stuck behind dma issues
            wjunk = cpool.tile([P, P], BF16)
            nc.vector.memset(wjunk[:], 0.25)
            ident = cpool.tile([P, P], F32)
            masks.make_identity(nc, ident[:])
            tri = cpool.tile([P, P], BF16)
            masks.make_upper_triangular(nc, tri[:], val=1.0, diag=True)

            # --- weights + x tiles ---
            if use_dr:
                wqk_sb = cpool.tile([P, EC2, 2, 2 * DH], FP8)
            else:
                wqk_sb = cpool.tile([P, EC, 2 * DH], FP8)
            wv_sb = cpool.tile([P, EC, DH], BF16)
            if has_padding:
                km_sb = cpool.tile([P, KC], F32)
                nc.sync.dma_start(km_sb[:], km_d.ap().transpose([1, 0]))

            xq_sb, xk_sb, xv_sb = {}, {}, {}

            def alloc_qk(s):
                shp = [P, EC2, 2, QW] if use_dr else [P, EC, QW]
                xq_sb[s] = xpool.tile(shp, FP8, tag="x", name=f"xq{s}")
                xk_sb[s] = xpool.tile(shp, FP8, tag="x", name=f"xk{s}")

            def alloc_v(s):
                xv_sb[(s, 0)] = xpool.tile([P, EC // 2, QW], BF16, tag="x",
                                           name=f"xv{s}a")
                xv_sb[(s, 1)] = xpool.tile([P, EC // 2, QW], BF16, tag="x",
                                           name=f"xv{s}b")

            def dma_x(eng, t, dram, s, half=None):
                rows = dram.ap()[s * P:(s + 1) * P]
                if half is None:
                    eng.dma_start(t[:], rows)
                else:
                    eng.dma_start(
                        t[:], rows[:, half * (EC // 2):(half + 1) * (EC // 2), :]
                    )

            # DMA sequencing entirely via x-pool buf reuse (4 bufs): a
            # tile's DMA start carries a WAR dep on the previous occupant's
            # proj reads. This is the only reliable transfer
            # prioritization — the rings round-robin across all in-flight
            # DMAs and the tile scheduler reorders plain issue order.
            # Waves: [q0 k0 q1 k1] -> v0 (after proj0) -> v1 (after proj1)
            # -> q2 k2 (after proj_v0) -> q3 k3 (after proj_v1) -> v2
            # (after proj_qk2) -> v3 (after proj_qk3).
            alloc_qk(0)
            alloc_qk(1)
            nc.scalar.dma_start(wqk_sb[:], wqk_d.ap())
            nc.scalar.dma_start(wv_sb[:], wv_d.ap())
            dma_x(nc.scalar, xq_sb[0], xq_d, 0)
            dma_x(nc.sync, xk_sb[0], xk_d, 0)
            dma_x(nc.sync, xq_sb[1], xq_d, 1)
            dma_x(nc.gpsimd, xk_sb[1], xk_d, 1)

            # --- PE warmup: HAM clock-gates the PE array to 1.2 GHz until
            # ~3.4us of sustained matmul activity ---
            if OPTS["pe_warm"]:
                wps = jpool.tile([P, QW], F32, tag="j", name="warmps")
                for _ in range(OPTS["pe_warm"]):
                    nc.tensor.matmul(
                        wps[:, :P], wjunk[:], wjunk[:],
                        start=True, stop=True, skip_group_check=True,
                    )

            qt = bigpool.tile([P, N], BF16, tag="qt")   # rows 0-63 QT, 64-127 dup
            kt = bigpool.tile([P, N], BF16, tag="kt")
            vt = bigpool.tile([DH, N], F32, tag="vt")
            v_sb = bigpool.tile([P, KC, DH + 1], BF16, tag="vn")

            def proj_qk(s):
                sl = slice(s * QW, (s + 1) * QW)
                for tname, x_t, big in (("q", xq_sb[s], qt), ("k", xk_sb[s], kt)):
                    ps = jpool.tile([P, QW], F32, tag="j", name=f"{tname}p{s}")
                    if use_dr:
                        for c in range(EC2):
                            nc.tensor.matmul(
                                ps[:],
                                wqk_sb[:, c],
                                x_t[:, c],
                                start=(c == 0),
                                stop=(c == EC2 - 1),
                                perf_mode=DR,
                            )
                    else:
                        for c in range(EC):
                            nc.tensor.matmul(
                                ps[:],
                                wqk_sb[:, c, :],
                                x_t[:, c, :],
                                start=(c == 0),
                                stop=(c == EC - 1),
                            )
                    nc.vector.tensor_copy(big[:, sl], ps[:])

            def proj_v(s):
                sl = slice(s * QW, (s + 1) * QW)
                ps = jpool.tile([P, QW], F32, tag="j", name=f"vp{s}")
                for c in range(EC):
                    nc.tensor.matmul(
                        ps[:DH, :],
                        wv_sb[:, c, :],
                        xv_sb[(s, c // 2)][:, c % 2, :],
                        start=(c == 0),
                        stop=(c == EC - 1),
                    )
                nc.vector.tensor_copy(vt[:, sl], ps[:DH, :])
                # V natural tiles: PE transpose + ones column (row-sums of P
                # come free as row 64 of the PV matmul)
                vtp = jpool.tile([P, NQB, DH + 1], F32, tag="j", name=f"vt{s}")
                for i in range(NQB):
                    j = s * NQB + i
                    nc.tensor.transpose(
                        vtp[:, i, :DH], vt[:, j * P:(j + 1) * P], ident[:DH, :DH]
                    )
                nc.vector.memset(vtp[:, :, DH], 1.0)
                nc.vector.tensor_copy(v_sb[:, s * NQB:(s + 1) * NQB, :], vtp[:])

            # --- attention, q-block outer; k-chunk pairs row-packed ---
            def emit_s_pair(qb, t, p_tiles):
                j0, j1 = 2 * t, 2 * t + 1
                s_ps = spool.tile([P, 2 * QW], F32, tag="s", name=f"s{qb}_{t}")
                # exp reads contiguously from q_off0; j1 computes full width
                # so no unwritten PSUM is read
                q_off0 = max(0, j0 * P - qb * QW) if causal else 0
                nc.tensor.matmul(
                    s_ps[:, q_off0:QW],
                    kt[0:DH, j0 * P:(j0 + 1) * P],
                    qt[0:DH, qb * QW + q_off0:(qb + 1) * QW],
                    start=True, stop=True,
                )
                nc.tensor.matmul(
                    s_ps[:, QW:],
                    kt[DH:P, j1 * P:(j1 + 1) * P],
                    qt[DH:P, qb * QW:(qb + 1) * QW],
                    start=True, stop=True,
                )
                p_sb = ppool.tile([P, 2 * QW], BF16, tag="p", name=f"p{qb}_{t}")
                nc.scalar.activation(
                    p_sb[:, q_off0:],
                    s_ps[:, q_off0:],
                    mybir.ActivationFunctionType.Exp,
                    scale=EXP_SCALE,
                )
                if causal:
                    # late q-blocks' diag masks go to the (by then idle)
                    # gpsimd engine to offload DVE
                    teng = nc.gpsimd if qb >= 2 else nc.vector
                    for idx, j in enumerate((j0, j1)):
                        if j // NQB == qb:
                            # diagonal 128x128 block: keep q_loc >= k_loc
                            lo = idx * QW + (j % NQB) * P
                            teng.tensor_mul(
                                p_sb[:, lo:lo + P], p_sb[:, lo:lo + P], tri[:]
                            )
                if has_padding:
                    for idx, j in enumerate((j0, j1)):
                        off = max(0, j * P - qb * QW) if causal else 0
                        nc.vector.tensor_scalar_mul(
                            p_sb[:, idx * QW + off:(idx + 1) * QW],
                            p_sb[:, idx * QW + off:(idx + 1) * QW],
                            km_sb[:, j:j + 1],
                        )
                p_tiles[t] = p_sb

            def emit_pv(qb, t, o_ps, p_tiles, first, last):
                p_sb = p_tiles.pop(t)
                for idx, j in enumerate((2 * t, 2 * t + 1)):
                    q_off = max(0, j * P - qb * QW) if causal else 0
                    nc.tensor.matmul(
                        o_ps[:, q_off:QW],
                        v_sb[:, j, :],
                        p_sb[:, idx * QW + q_off:(idx + 1) * QW],
                        start=(first and idx == 0),
                        stop=(last and idx == 1),
                    )

            def epilogue(qb, o_ps):
                oT = epipool.tile([DH + 1, QW], F32, tag="ot")
                nc.vector.tensor_copy(oT[:], o_ps[:])
                etp = jpool.tile([P, NQB, DH + 1], F32, tag="j", name=f"et{qb}")
                for i in range(NQB):
                    nc.tensor.transpose(
                        etp[:, i, :], oT[:, i * P:(i + 1) * P],
                        ident[:DH + 1, :DH + 1],
                    )
                recip = epipool.tile([P, NQB], F32, tag="recip")
                nc.vector.reciprocal(recip[:], etp[:, :, DH])
                o_sb = opool_sb.tile([P, NQB, DH], F32, tag="osb",
                                     name=f"osb{qb}")
                for i in range(NQB):
                    nc.vector.tensor_scalar_mul(
                        o_sb[:, i, :], etp[:, i, :DH], recip[:, i:i + 1]
                    )
                nc.sync.dma_start(
                    out_d.ap()[qb * P:(qb + 1) * P, :], o_sb[:]
                )

            # --- main emission: proj interleaved with q-block phases;
            # software-pipelined S/PV so PE work overlaps the wide exps ---
            def dma_v(s):
                alloc_v(s)
                dma_x(nc.sync, xv_sb[(s, 0)], xv_d, s, half=0)
                dma_x(nc.gpsimd, xv_sb[(s, 1)], xv_d, s, half=1)

            def dma_qk(s):
                alloc_qk(s)
                dma_x(nc.sync, xq_sb[s], xq_d, s)
                dma_x(nc.gpsimd, xk_sb[s], xk_d, s)

            def attn_qb(qb, npairs, t_projv, reverse, post_projv=None):
                order = list(range(npairs))
                if reverse:
                    order.reverse()
                o_ps = opool.tile([DH + 1, QW], F32, tag="o", name=f"o{qb}")
                p_tiles = {}
                for pos, t in enumerate(order):
                    emit_s_pair(qb, t, p_tiles)
                    if t == t_projv:
                        proj_v(qb)
                        if post_projv is not None:
                            post_projv()
                    if pos > 0:
                        emit_pv(qb, order[pos - 1], o_ps, p_tiles,
                                first=(pos == 1), last=False)
                emit_pv(qb, order[-1], o_ps, p_tiles,
                        first=(npairs == 1), last=True)
                epilogue(qb, o_ps)

            if causal:
                proj_qk(0)
                dma_v(0)      # v0 tiles reuse q0/k0 bufs -> gated on proj0
                proj_qk(1)
                dma_v(1)
                for qb in range(NQB):
                    npairs = 2 * qb + 2
                    t_projv = npairs - 1 if qb == 0 else 2 * qb
                    # q2/k2 ride on v0's bufs (gated on proj_v(0) reads);
                    # q3/k3 on v1's. last q-block runs pairs in reverse so
                    # its tail after the final exp is a mask-free PV
                    post = None
                    if qb == 0:
                        post = lambda: dma_qk(2)
                    elif qb == 1:
                        post = lambda: dma_qk(3)
                    attn_qb(qb, npairs, t_projv,
                            reverse=(qb == NQB - 1), post_projv=post)
                    if qb + 2 < NQB:
                        proj_qk(qb + 2)
                        dma_v(qb + 2)   # v2/v3 reuse q2/k2, q3/k3 bufs
            else:
                proj_qk(0)
                dma_v(0)
                proj_qk(1)
                dma_v(1)
                proj_v(0)
                dma_qk(2)
                proj_v(1)
                dma_qk(3)
                proj_qk(2)
                dma_v(2)
                proj_qk(3)
                dma_v(3)
                proj_v(2)
                proj_v(3)
                for qb in range(NQB):
                    attn_qb(qb, KC // 2, -1, reverse=False)

    nc.compile()
    return nc


def _get(causal: bool, has_padding: bool):
    key = (causal, has_padding)
    if key not in _BUILD_CACHE:
        _BUILD_CACHE[key] = _build(causal, has_padding)
    return _BUILD_CACHE[key]


def _pack_x(x_t: np.ndarray, dtype) -> np.ndarray:
    """[D, N] -> SBUF tile layout [(slice p), chunk, qw]."""
    return np.ascontiguousarray(
        x_t.reshape(EC, P, NQB, QW).transpose(2, 1, 0, 3)
        .reshape(NQB * P, EC, QW).astype(dtype)
    )


def _pack_x_dr(x_t: np.ndarray, dtype) -> np.ndarray:
    """[D, N] -> DoubleRow tile layout [(slice p), c, ko, qw],
    d = c*256 + ko*128 + ki."""
    return np.ascontiguousarray(
        x_t.reshape(EC2, 2, P, NQB, QW).transpose(3, 2, 0, 1, 4)
        .reshape(NQB * P, EC2, 2, QW).astype(dtype)
    )


def run(key_input, query_input, value_input, padding_mask, masked_attention,
        W_key, W_query=None, W_value=None, trace=False, **_ignored):
    key_input = np.asarray(key_input, dtype=np.float32)
    query_input = np.asarray(query_input, dtype=np.float32)
    value_input = np.asarray(value_input, dtype=np.float32)
    padding_mask = np.asarray(padding_mask)
    W_key = np.asarray(W_key, dtype=np.float32)

    B = key_input.shape[0]
    causal = bool(int(np.asarray(masked_attention)))
    has_padding = bool(padding_mask.any())
    nc = _get(causal, has_padding)

    bf = ml_dtypes.bfloat16
    f8 = ml_dtypes.float8_e4m3fn
    wcat = np.concatenate([W_key, W_key], axis=1) * WS
    if OPTS["use_dr"]:
        wqk = np.ascontiguousarray(
            wcat.reshape(EC2, 2, P, 2 * DH).transpose(2, 0, 1, 3).astype(f8)
        )
    else:
        wqk = np.ascontiguousarray(
            wcat.reshape(EC, P, 2 * DH).transpose(1, 0, 2).astype(f8)
        )
    wv = np.ascontiguousarray(
        W_key.reshape(EC, P, DH).transpose(1, 0, 2).astype(bf)
    )
    pack_qk = _pack_x_dr if OPTS["use_dr"] else _pack_x
    in_maps = []
    for b in range(B):
        m = {
            "xq": pack_qk(query_input[b].T, f8),
            "xk": pack_qk(key_input[b].T, f8),
            "xv": _pack_x(value_input[b].T, bf),
            "wqk": wqk,
            "wv": wv,
        }
        if has_padding:
            km = (~padding_mask[b].reshape(N)).astype(np.float32)
            m["kmask"] = np.ascontiguousarray(km.reshape(KC, P))
        in_maps.append(m)

    if trace:
        _ensure_ntff_hook()
    res = run_bass_kernel_spmd(nc, in_maps, core_ids=list(range(B)), trace=trace)
    outs = []
    for b in range(B):
        o = np.asarray(res.results[b]["out"])  # [(qb p), (i d)]
        o = o.reshape(NQB, P, NQB, DH).transpose(0, 2, 1, 3).reshape(N, DH)
        outs.append(o)
    out = np.stack(outs, axis=0)
    return out.astype(np.float32), res


def kernel(**inputs) -> np.ndarray:
    out, _ = run(**inputs)
    return out
